# revision 1
# baseline (speedup 1.0000x reference)
"""Policy-network kernel for Trainium2 (Bass/Tile), SPMD over 8 NeuronCores.

Strategy: data-parallel over batch B=128 -> 16 batches per core; all tables
and MLP weights replicated; no collectives. Heavy matmuls run as float32r
(full PE rate at N>=256). The per-(b,a) relation gather of the attention
output is fused on-chip via a one-hot matmul; rel/ent embedding row gathers
use indirect DMA.
"""

import numpy as np

# Problem dims (hardcoded per contract)
B, S, Dw, Dr, De, H, R, E, A = 128, 32, 300, 256, 256, 512, 512, 50000, 256
ACT = Dr + De          # 512
NCORES = 8
BL = B // NCORES       # 16 batches per core
BSL = BL * S           # 512 rows per core
NEG = -1e9

_CACHE = {}


def _build():
    import concourse.bass as bass
    import concourse.tile as tile
    from concourse import bacc, mybir

    f32 = mybir.dt.float32
    f32r = mybir.dt.float32r
    bf16 = mybir.dt.bfloat16
    i32 = mybir.dt.int32
    ts = bass.ts

    nc = bacc.Bacc("TRN2", target_bir_lowering=False, debug=False)

    def din(name, shape, dt=f32):
        return nc.dram_tensor(name, shape, dt, kind="ExternalInput").ap()

    xT = nc.dram_tensor("xT", [Dw, BSL], bf16, kind="ExternalInput").ap()
    w_step = nc.dram_tensor("w_step", [Dw, Dr], bf16, kind="ExternalInput").ap()
    b_step = din("b_step", [Dr, 1])
    relwT = nc.dram_tensor("relwT", [Dr, R], bf16, kind="ExternalInput").ap()
    mask_row = nc.dram_tensor("mask_row", [1, BSL], bf16, kind="ExternalInput").ap()
    iota4 = din("iota4", [128, 4])         # iota4[p,t] = t*128+p
    ridx_f = din("ridx_f", [BL, A])        # r_space as f32

    g_all_in = nc.dram_tensor("g_all_in", [128, 2 * BL * ACT], bf16, kind="ExternalInput").ap()
    phT = nc.dram_tensor("phT", [H, BL], bf16, kind="ExternalInput").ap()
    w1a = nc.dram_tensor("w1a", [H, ACT], bf16, kind="ExternalInput").ap()
    w1b = nc.dram_tensor("w1b", [Dr, ACT], bf16, kind="ExternalInput").ap()
    b1 = din("b1", [ACT, 1])
    w2 = nc.dram_tensor("w2", [ACT, ACT], bf16, kind="ExternalInput").ap()
    amask = nc.dram_tensor("amask", [128, 2 * BL], f32, kind="ExternalInput").ap()
    pairmat = nc.dram_tensor("pairmat", [2 * BL, 2 * BL], f32r, kind="ExternalInput").ap()
    ones_col = nc.dram_tensor("ones_col", [128, 256], f32r, kind="ExternalInput").ap()
    ident_in = nc.dram_tensor("ident_in", [128, 128], f32r, kind="ExternalInput").ap()
    ident_in_bf = nc.dram_tensor("ident_in_bf", [128, 128], bf16, kind="ExternalInput").ap()
    ones_row = nc.dram_tensor("ones_row", [1, 128], bf16, kind="ExternalInput").ap()

    out_dram = nc.dram_tensor("out", [BL, A], f32, kind="ExternalOutput").ap()


    KD = [(0, 128), (128, 256), (256, 300)]   # Dw K-tiles

    from concourse.masks import make_identity

    with tile.TileContext(nc) as tc:
        with (
            tc.tile_pool(name="const", bufs=1) as cpool,
            tc.tile_pool(name="work", bufs=2) as wpool,
            tc.tile_pool(name="perb", bufs=4) as bpool,
            tc.tile_pool(name="ps_big", bufs=2, space="PSUM") as ps_big,
            tc.tile_pool(name="ps_h2", bufs=2, space="PSUM") as ps_h2p,
            tc.tile_pool(name="ps_med", bufs=2, space="PSUM") as ps_med,
            tc.tile_pool(name="ps_small", bufs=2, space="PSUM") as ps_small,
        ):
            # ---- constants to SBUF (critical-path order; all issues before compute) ----
            def load_const(tag, src, p0, p1, cols, dt=f32, eng=None):
                t = cpool.tile([p1 - p0, cols], dt, tag=tag)
                (eng or nc.sync).dma_start(t[:], src[p0:p1, :])
                return t

            def r(ap):
                return ap

            # sync queue: front-end critical path first, k-interleaved so the
            # k=0 matmul can start as soon as its own tiles land
            w_step_sb, xT_sb = [], []
            for k, (a, b_) in enumerate(KD):
                w_step_sb.append(load_const(f"ws{k}", w_step, a, b_, Dr, bf16))
                xT_sb.append(load_const(f"xt{k}", xT, a, b_, BSL, bf16))
            b_step_sb = [load_const(f"bs{k}", b_step, k * 128, (k + 1) * 128, 1, f32) for k in range(2)]
            relwT_sb = [load_const(f"rw{k}", relwT, k * 128, (k + 1) * 128, R, bf16) for k in range(2)]
            mask_sb = cpool.tile([1, BSL], bf16, tag="mask")
            nc.sync.dma_start(mask_sb[:], mask_row[:])
            ones1 = cpool.tile([1, 128], bf16, tag="ones1")
            nc.sync.dma_start(ones1[:], ones_row[:])
            iota_sb = cpool.tile([128, 4], f32, tag="iota")
            nc.sync.dma_start(iota_sb[:], iota4[:])
            idxall = cpool.tile([128, BL, A], f32, tag="idxall")
            nc.sync.dma_start(idxall[:].rearrange("p b a -> p (b a)"),
                              ridx_f[:].rearrange("b a -> (b a)").unsqueeze(0)
                              .partition_broadcast(128))
            # scalar queue: MLP-phase weights + tail consts (all issues up-front)
            ident = cpool.tile([128, 128], bf16, tag="ident")
            nc.scalar.dma_start(ident[:], ident_in_bf[:])
            w1b_sb = [load_const(f"w1b{k}", w1b, k * 128, (k + 1) * 128, ACT, bf16, nc.scalar) for k in range(2)]
            w1a_sb = [load_const(f"w1a{k}", w1a, k * 128, (k + 1) * 128, ACT, bf16, nc.scalar) for k in range(4)]
            phT_sb = [load_const(f"ph{k}", phT, k * 128, (k + 1) * 128, BL, bf16, nc.scalar) for k in range(4)]
            b1_sb = [load_const(f"b1{k}", b1, k * 128, (k + 1) * 128, 1, f32, nc.scalar) for k in range(4)]
            w2_sb = [load_const(f"w2{k}", w2, k * 128, (k + 1) * 128, ACT, bf16, nc.scalar) for k in range(4)]
            # pre-gathered action embeddings: 4 chunks split across both queues
            g_all = cpool.tile([128, 2 * BL, ACT], bf16, tag="g_all")
            g_flat = g_all[:].rearrange("p j d -> p (j d)")
            CH = 2 * BL * ACT // 4
            for ch in range(4):
                eng = nc.sync if ch % 2 == 0 else nc.scalar
                eng.dma_start(g_flat[:, ch * CH:(ch + 1) * CH],
                              g_all_in[:, ch * CH:(ch + 1) * CH])
            amask_p = cpool.tile([128, 2 * BL], f32, tag="amask_p")
            nc.scalar.dma_start(amask_p[:], amask[:])
            pairmat_sb = cpool.tile([2 * BL, 2 * BL], f32r, tag="pairmat")
            nc.scalar.dma_start(pairmat_sb[:], pairmat[:])
            ones_col_sb = cpool.tile([128, 256], f32r, tag="ones_col")
            nc.scalar.dma_start(ones_col_sb[:], ones_col[:])
            ident_f = cpool.tile([128, 128], f32r, tag="ident_f")
            nc.scalar.dma_start(ident_f[:], ident_in[:])
            Elg = cpool.tile([128, 2 * BL], f32, tag="Elg")
            # warm the ACT function table after all DMA issues
            act_warm = cpool.tile([128, 4], f32, tag="act_warm")
            nc.scalar.activation(act_warm[:], iota_sb[:],
                                 bass.mybir.ActivationFunctionType.Exp, scale=0.0)

            # ---- saqT = tanh(W_step.T @ xT + b_step)  [2][128, BSL] ----
            saqT_sb = []
            for t in range(2):
                ps = ps_big.tile([128, BSL], f32, tag="big")
                for k in range(3):
                    nc.tensor.matmul(ps[:], r(w_step_sb[k][:, ts(t, 128)]), r(xT_sb[k][:]),
                                     start=(k == 0), stop=(k == 2))
                sb = cpool.tile([128, BSL], bf16, tag=f"saqT{t}")
                nc.scalar.activation(sb[:], ps[:], bass.mybir.ActivationFunctionType.Tanh,
                                     bias=b_step_sb[t][:])
                saqT_sb.append(sb)

            # ---- scores + masked softmax per r-tile -> alpha [4][128, BL, S] ----
            alpha_sb = []
            for rt in range(4):
                ps = ps_big.tile([128, BSL], f32, tag="big")
                for k in range(2):
                    nc.tensor.matmul(ps[:], r(relwT_sb[k][:, ts(rt, 128)]), r(saqT_sb[k][:]),
                                     start=(k == 0), stop=False)
                nc.tensor.matmul(ps[:], r(ones1[:]), r(mask_sb[:]), start=False, stop=True)
                al32 = wpool.tile([128, BL, S], f32, tag="al32")
                nc.scalar.activation(al32[:].rearrange("p b s -> p (b s)"), ps[:],
                                     bass.mybir.ActivationFunctionType.Exp)
                sums = wpool.tile([128, BL], f32, tag="sums")
                nc.vector.tensor_reduce(sums[:], al32[:], axis=bass.mybir.AxisListType.X,
                                        op=bass.mybir.AluOpType.add)
                rec = wpool.tile([128, BL], f32, tag="rec")
                nc.vector.reciprocal(rec[:], sums[:])
                al = cpool.tile([128, BL, S], bf16, tag=f"alpha{rt}")
                nc.vector.tensor_mul(al[:], al32[:],
                                     rec[:].unsqueeze(2).to_broadcast((128, BL, S)))
                alpha_sb.append(al)

            # ---- saq natural per b: [32, BL, Dr] via PE transpose ----
            saq_nat = cpool.tile([32, BL, Dr], bf16, tag="saq_nat")
            for b in range(BL):
                ps = ps_small.tile([32, Dr], f32, tag="small")
                psb = ps[:].bitcast(bf16)
                for t in range(2):
                    nc.tensor.transpose(psb[:, ts(t, 128)],
                                        saqT_sb[t][:, b * S:(b + 1) * S],
                                        ident[:])
                nc.scalar.copy(saq_nat[:, b, :], psb[:, 0:Dr])

            # ---- ph contribution + fused h1 bias: biasT[t][:,b] = (ph@W1a)T + b1 ----
            biasT = cpool.tile([128, 4, BL], f32, tag="biasT")
            for t in range(4):
                ps = ps_med.tile([128, BL], f32, tag="med")
                for k in range(4):
                    nc.tensor.matmul(ps[:], r(w1a_sb[k][:, ts(t, 128)]), r(phT_sb[k][:]),
                                     start=(k == 0), stop=(k == 3))
                nc.vector.tensor_scalar_add(biasT[:, t, :], ps[:], b1_sb[t][:])

            # ---- per-batch pipeline, stage-major over groups of 4 ----
            onehots, alpha_gTs, raq_gTs, h1Ts = {}, {}, {}, {}
            for g in range(BL // 4):
                bs = range(4 * g, 4 * g + 4)
                for b in bs:
                    onehot = bpool.tile([128, 4, A], bf16, tag="onehot", bufs=5)
                    nc.vector.tensor_tensor(
                        onehot[:],
                        idxall[:, b, :].unsqueeze(1).to_broadcast((128, 4, A)),
                        iota_sb[:].unsqueeze(2).to_broadcast((128, 4, A)),
                        op=bass.mybir.AluOpType.is_equal)
                    onehots[b] = onehot
                for b in bs:
                    ps_ag = ps_small.tile([S, A], f32, tag="small")
                    for rt in range(4):
                        nc.tensor.matmul(ps_ag[:], r(alpha_sb[rt][:, b, :]),
                                         r(onehots[b][:, rt, :]),
                                         start=(rt == 0), stop=(rt == 3))
                    alpha_gT = bpool.tile([S, A], bf16, tag="alpha_gT", bufs=6)
                    nc.vector.tensor_copy(alpha_gT[:], ps_ag[:])
                    alpha_gTs[b] = alpha_gT
                for b in bs:
                    raq_gT = bpool.tile([128, 2, A], bf16, tag="raq_gT", bufs=5)
                    ps_rq = ps_big.tile([128, 2 * A], f32, tag="big")
                    for dt_ in range(2):
                        nc.tensor.matmul(ps_rq[:, ts(dt_, A)], r(saq_nat[:, b, ts(dt_, 128)]),
                                         r(alpha_gTs[b][:]), start=True, stop=True)
                    nc.vector.tensor_copy(raq_gT[:], ps_rq[:].rearrange("p (d a) -> p d a", a=A))
                    raq_gTs[b] = raq_gT
                for b in bs:
                    h1T = bpool.tile([128, 4, A], bf16, tag="h1T", bufs=5)
                    for t in range(4):
                        ps_h1 = ps_med.tile([128, A], f32, tag="med")
                        for k in range(2):
                            nc.tensor.matmul(ps_h1[:], r(w1b_sb[k][:, ts(t, 128)]),
                                             r(raq_gTs[b][:, k, :]), start=(k == 0), stop=(k == 1))
                        nc.scalar.activation(h1T[:, t, :], ps_h1[:],
                                             bass.mybir.ActivationFunctionType.Relu,
                                             bias=biasT[:, t, b:b + 1])
                    h1Ts[b] = h1T
                for b in bs:
                    for c in range(2):
                        ps_h2 = ps_h2p.tile([128, ACT], f32, tag="h2")
                        for k in range(4):
                            nc.tensor.matmul(ps_h2[:], r(h1Ts[b][:, k, ts(c, 128)]), r(w2_sb[k][:]),
                                             start=(k == 0), stop=(k == 3))
                        scratch = bpool.tile([128, ACT], f32, tag="scratch", bufs=3)
                        nc.vector.tensor_mul(scratch[:], ps_h2[:], g_all[:, b * 2 + c, :])
                        trash = bpool.tile([128, ACT], bf16, tag="trash", bufs=2)
                        nc.scalar.activation(trash[:], scratch[:],
                                             bass.mybir.ActivationFunctionType.Identity,
                                             accum_out=Elg[:, b * 2 + c:b * 2 + c + 1])
            # ---- final softmax fully on-chip, partition layout ----
            Elgm = wpool.tile([128, 2 * BL], f32, tag="Elgm")
            nc.gpsimd.tensor_add(Elgm[:], Elg[:], amask_p[:])
            Eexp = wpool.tile([128, 2 * BL], f32r, tag="Eexp")
            nc.scalar.activation(Eexp[:], Elgm[:], bass.mybir.ActivationFunctionType.Exp)
            ps_s = ps_small.tile([32, 256], f32, tag="small")
            nc.tensor.matmul(ps_s[:], Eexp[:], ones_col_sb[:],
                             start=True, stop=True)
            s_sb = wpool.tile([32, 256], f32r, tag="s_sb")
            nc.vector.tensor_copy(s_sb[:], ps_s[:])
            ps_s2 = ps_small.tile([32, 256], f32, tag="small")
            nc.tensor.matmul(ps_s2[:], pairmat_sb[:], s_sb[:], start=True, stop=True)
            rec_sb = wpool.tile([32, 1], f32, tag="rec_sb")
            nc.vector.reciprocal(rec_sb[:], ps_s2[:, 0:1])
            ps_ET = ps_small.tile([32, 256], f32, tag="small")
            nc.tensor.transpose(ps_ET[:, 0:128].bitcast(f32r), Eexp[:], ident_f[:])
            osb = wpool.tile([32, 128], f32, tag="osb")
            nc.vector.tensor_scalar_mul(osb[:], ps_ET[:, 0:128], rec_sb[:])
            nc.sync.dma_start(out_dram[:].rearrange("b (c p) -> (b c) p", c=2), osb[:])
    nc.compile()
    return nc


def _host_prep(inputs):
    """Build the 8 per-core input maps from full inputs."""
    x = np.asarray(inputs["transformer_output"], np.float32)
    qmask = np.asarray(inputs["question_mask"])
    W_step = np.ascontiguousarray(np.asarray(inputs["W_step"], np.float32))
    b_step = np.asarray(inputs["b_step"], np.float32).reshape(Dr, 1)
    w_att = np.asarray(inputs["w_att"], np.float32)
    rel_emb = np.ascontiguousarray(np.asarray(inputs["rel_emb"], np.float32))
    ent_emb = np.ascontiguousarray(np.asarray(inputs["ent_emb"], np.float32))
    ph = np.asarray(inputs["path_hidden"], np.float32)
    W1 = np.asarray(inputs["W1"], np.float32)
    b1 = np.asarray(inputs["b1"], np.float32).reshape(ACT, 1)
    W2 = np.ascontiguousarray(np.asarray(inputs["W2"], np.float32))
    b2 = np.asarray(inputs["b2"], np.float32).reshape(1, ACT)
    r_space = np.asarray(inputs["r_space"], np.int32)
    e_space = np.asarray(inputs["e_space"], np.int32)
    action_mask = np.asarray(inputs["action_mask"], np.float32)

    relwT = np.ascontiguousarray((rel_emb * w_att[None, :]).T)   # [Dr, R]
    w1a = np.ascontiguousarray(W1[:H])
    w1b = np.ascontiguousarray(W1[H:])
    iota4 = (np.arange(128, dtype=np.float32)[:, None]
             + 128.0 * np.arange(4, dtype=np.float32)[None, :])
    iota4 = np.ascontiguousarray(iota4)

    import ml_dtypes
    BF = ml_dtypes.bfloat16
    W1a_bf = np.ascontiguousarray(w1a.astype(BF))
    rel_bf = np.ascontiguousarray(rel_emb.astype(BF))
    W_step_bf = np.ascontiguousarray(W_step.astype(BF))
    relwT_bf = np.ascontiguousarray(relwT.astype(BF))
    ent_bf = np.ascontiguousarray(ent_emb.astype(BF))
    W1b_bf = np.ascontiguousarray(w1b.astype(BF))
    W2_bf = np.ascontiguousarray(W2.astype(BF))
    c_rel = rel_emb @ b2[0, :Dr]
    c_ent = ent_emb @ b2[0, Dr:]
    jj = np.arange(2 * BL)
    pairmat_np = np.ascontiguousarray((jj[:, None] // 2 == jj[None, :] // 2).astype(np.float32))
    ones_col_np = np.ones((128, 256), np.float32)
    ident_np = np.eye(128, dtype=np.float32)
    import ml_dtypes as _mld
    ident_bf_np = np.eye(128, dtype=np.float32).astype(_mld.bfloat16)
    in_maps = []
    for i in range(NCORES):
        b0, b1_ = i * BL, (i + 1) * BL
        xs = x[b0:b1_].reshape(BSL, Dw)
        mrow = np.where(qmask[b0:b1_].reshape(1, BSL), np.float32(NEG), np.float32(0.0))
        amask_add = np.where(action_mask[b0:b1_] > 0, np.float32(0.0), np.float32(NEG))
        amask_add = amask_add + c_rel[r_space[b0:b1_]] + c_ent[e_space[b0:b1_]]
        amask_p = np.ascontiguousarray(
            amask_add.reshape(BL, 2, 128).transpose(2, 0, 1).reshape(128, 2 * BL))
        grel = rel_bf[r_space[b0:b1_]]
        gent = ent_bf[e_space[b0:b1_]]
        g_np = np.empty((128, 2 * BL, ACT), rel_bf.dtype)
        g_np[:, :, :Dr] = grel.reshape(BL, 2, 128, Dr).transpose(2, 0, 1, 3).reshape(128, 2 * BL, Dr)
        g_np[:, :, Dr:] = gent.reshape(BL, 2, 128, De).transpose(2, 0, 1, 3).reshape(128, 2 * BL, De)
        g_np = np.ascontiguousarray(g_np.reshape(128, 2 * BL * ACT))
        in_maps.append({
            "xT": np.ascontiguousarray(xs.T.astype(BF)),
            "w_step": W_step_bf,
            "b_step": b_step,
            "relwT": relwT_bf,
            "mask_row": np.ascontiguousarray(mrow.astype(BF)),
            "iota4": iota4,
            "ridx_f": np.ascontiguousarray(r_space[b0:b1_].astype(np.float32)),
            "g_all_in": g_np,
            "_r_space": r_space[b0:b1_],
            "phT": np.ascontiguousarray(ph[b0:b1_].T.astype(BF)),
            "w1a": W1a_bf,
            "w1b": W1b_bf,
            "b1": b1,
            "w2": W2_bf,
            "amask": amask_p,
            "pairmat": pairmat_np,
            "ones_col": ones_col_np,
            "ident_in": ident_np,
            "ident_in_bf": ident_bf_np,
            "ones_row": np.ones((1, 128), BF),
        })
    return in_maps


def kernel(**inputs):
    from concourse.bass_utils import run_bass_kernel_spmd

    if "nc" not in _CACHE:
        _CACHE["nc"] = _build()
    nc = _CACHE["nc"]
    in_maps = _host_prep(inputs)
    res = run_bass_kernel_spmd(nc, in_maps, list(range(NCORES)))
    return np.concatenate([res.results[i]["out"] for i in range(NCORES)], axis=0)

